# revision 4
# baseline (speedup 1.0000x reference)
"""MoE routing kernel (top-2 + bottom-2 of 8 experts) for 8 Trainium2 NeuronCores.

Strategy: data-parallel over B (B=8 -> one batch element per core, no
collectives). Each core runs the dense per-expert 2-layer MLP with layernorms
for its 4096 tokens, computes gating on-device in fp32, and combines experts
with softmax weights folded into the layernorm-2 affine. GEMMs run in bf16
with fp32 PSUM accumulation; all layernorm statistics are fp32.
"""

import os
import sys
from functools import lru_cache

import numpy as np

sys.path.insert(0, "/opt/trn_rl_repo")

import concourse.bacc as bacc  # noqa: E402
import concourse.bass as bass  # noqa: E402
import concourse.tile as tile  # noqa: E402
from concourse import mybir  # noqa: E402
from concourse.bass_utils import run_bass_kernel_spmd  # noqa: E402
from concourse.masks import make_identity  # noqa: E402

F32 = mybir.dt.float32
BF16 = mybir.dt.bfloat16
BF16_NP = mybir.dt.np(mybir.dt.bfloat16)

B, N, D, E = 8, 4096, 512, 8
P = 128              # partitions / token tile
KC = D // P          # 4 contraction chunks
NT = N // P          # 32 token tiles per core
LN_EPS = 1e-5
AF = mybir.ActivationFunctionType
ALU = mybir.AluOpType
AX = mybir.AxisListType


def _build(trivia):
    """Build the SPMD Bass program. `trivia` is a dict of bools saying which
    optional params (biases / ln affine params) are trivial (zeros/ones) and
    can be folded away."""
    nc = bacc.Bacc("TRN2", target_bir_lowering=False, debug=False, num_devices=8)

    # ---- dram parameters (per-core shards, host-prepped layouts) ----
    xt_f = nc.declare_dram_parameter("xt_f", [P, KC, N], F32, isOutput=False)
    xt_b = nc.declare_dram_parameter("xt_b", [P, KC, N], BF16, isOutput=False)
    x_f = nc.declare_dram_parameter("x_f", [N, D], F32, isOutput=False)
    w1_d = nc.declare_dram_parameter("w1", [P, E, KC, D], BF16, isOutput=False)
    w2_d = nc.declare_dram_parameter("w2", [P, E, KC, D], BF16, isOutput=False)
    wg_d = nc.declare_dram_parameter("wg", [P, KC, E], F32, isOutput=False)
    bg_d = nc.declare_dram_parameter("bg", [1, E], F32, isOutput=False)
    opt = {}
    for name in ("b1", "ln1_w", "ln1_b", "b2", "ln2_w", "ln2_b"):
        if not trivia[name]:
            opt[name] = nc.declare_dram_parameter(name, [E, D], F32, isOutput=False)

    out_d = nc.declare_dram_parameter("out", [N, D], F32, isOutput=True)
    otop_d = nc.declare_dram_parameter("out_top", [N, D], F32, isOutput=True)
    obot_d = nc.declare_dram_parameter("out_bot", [N, D], F32, isOutput=True)
    ss_d = nc.declare_dram_parameter("ss", [1, 1], F32, isOutput=True)

    def bcast(dram_ap, part_rows, free):
        # AP that reads one dram row broadcast across `part_rows` partitions
        return bass.AP(
            tensor=dram_ap.tensor,
            offset=dram_ap.offset,
            ap=[[0, part_rows]] + list(dram_ap.ap[1:]),
        )

    with tile.TileContext(nc) as tc, \
            tc.tile_pool(name="consts", bufs=1) as consts, \
            tc.tile_pool(name="wpool", bufs=1) as wpool, \
            tc.tile_pool(name="route", bufs=3) as route, \
            tc.tile_pool(name="wall", bufs=1) as wall, \
            tc.tile_pool(name="stage", bufs=2) as stage, \
            tc.tile_pool(name="work", bufs=3) as work, \
            tc.tile_pool(name="stats", bufs=2) as stats_p, \
            tc.tile_pool(name="outs", bufs=3) as outs_p, \
            tc.tile_pool(name="psum_h", bufs=2, space="PSUM") as psum_h, \
            tc.tile_pool(name="psum_t", bufs=1, space="PSUM") as psum_t, \
            tc.tile_pool(name="psum_z", bufs=2, space="PSUM") as psum_z, \
            tc.tile_pool(name="psum_a", bufs=1, space="PSUM") as psum_a:

        # ---------------- constants & resident weights ----------------
        ident = consts.tile([P, P], BF16)
        make_identity(nc, ident)
        eps_t = consts.tile([P, 1], F32)
        nc.vector.memset(eps_t, LN_EPS)
        ones_t = consts.tile([P, 1], F32)
        nc.vector.memset(ones_t, 1.0)
        bgbc = consts.tile([P, E], F32)
        nc.gpsimd.dma_start(out=bgbc, in_=bcast(bg_d[:], P, E))

        w1sb = wpool.tile([P, E, KC, D], BF16)
        nc.sync.dma_start(out=w1sb, in_=w1_d[:])
        w2sb = wpool.tile([P, E, KC, D], BF16)
        nc.sync.dma_start(out=w2sb, in_=w2_d[:])
        xtb = wpool.tile([P, KC, N], BF16)
        nc.sync.dma_start(out=xtb, in_=xt_b[:])
        wgsb = consts.tile([P, KC, E], F32)
        nc.sync.dma_start(out=wgsb, in_=wg_d[:])

        optbc = {}
        for name, prm in opt.items():
            t = wpool.tile([P, E, D], F32, tag=f"opt_{name}")
            for e in range(E):
                nc.gpsimd.dma_start(out=t[:, e, :], in_=bcast(prm[e : e + 1, :], P, D))
            optbc[name] = t

        ss_all = wall.tile([P, NT], F32)
        nc.vector.memset(ss_all, 0.0)
        wtop_all = wall.tile([P, NT, E], F32)
        wbot_all = wall.tile([P, NT, E], F32)

        # ---------------- gating (fp32, exact) ----------------
        with nc.named_scope("gating"):
            for t in range(NT):
                xg = route.tile([P, KC, P], F32, tag="xg")
                nc.gpsimd.dma_start(out=xg, in_=xt_f[:, :, t * P : (t + 1) * P])
                ps = psum_h.tile([P, E], F32, tag="h")
                for kk in range(KC):
                    nc.tensor.matmul(
                        ps, xg[:, kk, :], wgsb[:, kk, :],
                        start=(kk == 0), stop=(kk == KC - 1),
                    )
                sc = route.tile([P, E], F32, tag="sc")
                nc.vector.tensor_tensor(sc, ps, bgbc, op=ALU.add)
                srt = route.tile([P, E], F32, tag="srt")
                nc.vector.max(srt, sc)
                # softmax over a pair == sigmoid of the score difference
                dif = route.tile([P, 4], F32, tag="dif")
                nc.vector.tensor_tensor(dif[:, 0:1], srt[:, 1:2], srt[:, 0:1], op=ALU.subtract)
                nc.vector.tensor_tensor(dif[:, 1:2], srt[:, 6:7], srt[:, 7:8], op=ALU.subtract)
                nc.vector.tensor_tensor(dif[:, 2:3], srt[:, 0:1], srt[:, 1:2], op=ALU.subtract)
                nc.vector.tensor_tensor(dif[:, 3:4], srt[:, 7:8], srt[:, 6:7], op=ALU.subtract)
                sig = route.tile([P, 4], F32, tag="sig")
                nc.scalar.activation(sig, dif, AF.Sigmoid)
                m = route.tile([P, 4, E], F32, tag="m")
                # one-hot masks for the argmax/arg2nd/argmin/arg2nd-min experts
                for i, col in enumerate((0, 1, 7, 6)):
                    nc.vector.tensor_scalar(
                        m[:, i, :], sc, scalar1=srt[:, col : col + 1],
                        scalar2=None, op0=ALU.is_equal,
                    )
                # weights: top1 <- sig[2], top2 <- sig[0]; bot1 <- sig[3], bot2 <- sig[1]
                for i, s in ((0, 2), (1, 0), (2, 3), (3, 1)):
                    nc.vector.tensor_scalar(
                        m[:, i, :], m[:, i, :], scalar1=sig[:, s : s + 1],
                        scalar2=None, op0=ALU.mult,
                    )
                nc.vector.tensor_tensor(wtop_all[:, t, :], m[:, 0, :], m[:, 1, :], op=ALU.add)
                nc.vector.tensor_tensor(wbot_all[:, t, :], m[:, 2, :], m[:, 3, :], op=ALU.add)

        # ---------------- main loop ----------------
        for t in range(NT):
            tok = slice(t * P, (t + 1) * P)
            with nc.named_scope(f"tile{t}"):
                # ---- layer 1 matmuls + stats ----
                st1 = stats_p.tile([P, E, 6], F32, tag="st1")
                hsb = []
                for e in range(E):
                    ph = psum_h.tile([P, D], F32, tag="h")
                    for kk in range(KC):
                        nc.tensor.matmul(
                            ph, xtb[:, kk, tok], w1sb[:, e, kk, :],
                            start=(kk == 0), stop=(kk == KC - 1),
                        )
                    if "b1" in optbc:
                        nc.vector.tensor_tensor(ph, ph, optbc["b1"][:, e, :], op=ALU.add)
                    nc.vector.bn_stats(st1[:, e, :], ph)
                    h = stage.tile([P, D], BF16, tag=f"h{e}")
                    nc.scalar.copy(h, ph)
                    hsb.append(h)
                # ---- batched LN1 scalars ----
                mv1 = stats_p.tile([P, E, 2], F32, tag="mv1")
                for e in range(E):
                    nc.vector.bn_aggr(mv1[:, e, :], st1[:, e, :])
                r1 = stats_p.tile([P, E], F32, tag="r1")
                nc.scalar.activation(r1, mv1[:, :, 1], AF.Sqrt, bias=eps_t)
                nc.vector.reciprocal(r1, r1)
                nm1 = stats_p.tile([P, E], F32, tag="nm1")
                nc.vector.tensor_tensor(nm1, mv1[:, :, 0], r1, op=ALU.mult)
                nc.vector.tensor_scalar(nm1, nm1, scalar1=-1.0, scalar2=None, op0=ALU.mult)
                # ---- normalize+relu, transpose, layer 2 ----
                st2 = stats_p.tile([P, E, 6], F32, tag="st2")
                zsb = []
                for e in range(E):
                    g = work.tile([P, D], BF16, tag="g")
                    if trivia["ln1_w"] and trivia["ln1_b"]:
                        nc.scalar.activation(
                            g, hsb[e], AF.Relu,
                            bias=nm1[:, e : e + 1], scale=r1[:, e : e + 1],
                        )
                    else:
                        a = work.tile([P, D], BF16, tag="ga")
                        nc.scalar.activation(
                            a, hsb[e], AF.Copy,
                            bias=nm1[:, e : e + 1], scale=r1[:, e : e + 1],
                        )
                        if not trivia["ln1_w"]:
                            nc.vector.tensor_tensor(a, a, optbc["ln1_w"][:, e, :], op=ALU.mult)
                        if not trivia["ln1_b"]:
                            nc.vector.tensor_tensor(a, a, optbc["ln1_b"][:, e, :], op=ALU.add)
                        nc.vector.tensor_scalar(g, a, scalar1=0.0, scalar2=None, op0=ALU.max)
                    pt = psum_t.tile([P, KC, P], BF16, tag="t")
                    for c in range(KC):
                        nc.tensor.transpose(pt[:, c, :], g[:, c * P : (c + 1) * P], ident)
                    gt = work.tile([P, KC, P], BF16, tag="gt")
                    if e % 2 == 0:
                        nc.vector.tensor_copy(gt, pt)
                    else:
                        nc.scalar.copy(gt, pt)
                    pz = psum_z.tile([P, D], F32, tag="z")
                    for kk in range(KC):
                        nc.tensor.matmul(
                            pz, gt[:, kk, :], w2sb[:, e, kk, :],
                            start=(kk == 0), stop=(kk == KC - 1),
                        )
                    if "b2" in optbc:
                        nc.vector.tensor_tensor(pz, pz, optbc["b2"][:, e, :], op=ALU.add)
                    nc.vector.bn_stats(st2[:, e, :], pz)
                    z = stage.tile([P, D], BF16, tag=f"z{e}")
                    nc.scalar.copy(z, pz)
                    zsb.append(z)
                # ---- batched LN2 scalars, fold in gate weights ----
                mv2 = stats_p.tile([P, E, 2], F32, tag="mv2")
                for e in range(E):
                    nc.vector.bn_aggr(mv2[:, e, :], st2[:, e, :])
                r2 = stats_p.tile([P, E], F32, tag="r2")
                nc.scalar.activation(r2, mv2[:, :, 1], AF.Sqrt, bias=eps_t)
                nc.vector.reciprocal(r2, r2)
                stt = stats_p.tile([P, E], F32, tag="stt")
                nc.vector.tensor_tensor(stt, r2, wtop_all[:, t, :], op=ALU.mult)
                stb = stats_p.tile([P, E], F32, tag="stb")
                nc.vector.tensor_tensor(stb, r2, wbot_all[:, t, :], op=ALU.mult)
                # ---- weighted combine via identity-matmul PSUM accumulation ----
                pat = psum_a.tile([P, D], F32, tag="at")
                pab = psum_a.tile([P, D], F32, tag="ab")
                for e in range(E):
                    yt = work.tile([P, D], BF16, tag="yt")
                    nc.vector.tensor_scalar(
                        yt, zsb[e], scalar1=mv2[:, e, 0:1], scalar2=stt[:, e : e + 1],
                        op0=ALU.subtract, op1=ALU.mult,
                    )
                    if not trivia["ln2_w"]:
                        nc.vector.tensor_tensor(yt, yt, optbc["ln2_w"][:, e, :], op=ALU.mult)
                    if not trivia["ln2_b"]:
                        tmp = work.tile([P, D], F32, tag="l2btmp")
                        nc.vector.tensor_scalar(
                            tmp, optbc["ln2_b"][:, e, :],
                            scalar1=wtop_all[:, t, e : e + 1], scalar2=None, op0=ALU.mult,
                        )
                        nc.vector.tensor_tensor(yt, yt, tmp, op=ALU.add)
                    nc.tensor.matmul(pat, ident, yt, start=(e == 0), stop=(e == E - 1))
                    yb = work.tile([P, D], BF16, tag="yb")
                    nc.vector.tensor_scalar(
                        yb, zsb[e], scalar1=mv2[:, e, 0:1], scalar2=stb[:, e : e + 1],
                        op0=ALU.subtract, op1=ALU.mult,
                    )
                    if not trivia["ln2_w"]:
                        nc.vector.tensor_tensor(yb, yb, optbc["ln2_w"][:, e, :], op=ALU.mult)
                    if not trivia["ln2_b"]:
                        tmp = work.tile([P, D], F32, tag="l2btmp")
                        nc.vector.tensor_scalar(
                            tmp, optbc["ln2_b"][:, e, :],
                            scalar1=wbot_all[:, t, e : e + 1], scalar2=None, op0=ALU.mult,
                        )
                        nc.vector.tensor_tensor(yb, yb, tmp, op=ALU.add)
                    nc.tensor.matmul(pab, ident, yb, start=(e == 0), stop=(e == E - 1))
                # ---- finalize tile ----
                ot = outs_p.tile([P, D], F32, tag="ot")
                nc.scalar.copy(ot, pat)
                ob = outs_p.tile([P, D], F32, tag="ob")
                nc.vector.tensor_copy(ob, pab)
                nc.sync.dma_start(out=otop_d[tok, :], in_=ot)
                nc.sync.dma_start(out=obot_d[tok, :], in_=ob)
                df = outs_p.tile([P, D], F32, tag="df")
                nc.vector.tensor_tensor(df, ot, ob, op=ALU.subtract)
                sq = outs_p.tile([P, D], BF16, tag="sq")
                nc.scalar.activation(sq, df, AF.Square, accum_out=ss_all[:, t : t + 1])
                xr = outs_p.tile([P, D], F32, tag="xr")
                nc.gpsimd.dma_start(out=xr, in_=x_f[tok, :])
                oo = outs_p.tile([P, D], F32, tag="oo")
                nc.vector.tensor_tensor(oo, ot, xr, op=ALU.add)
                nc.sync.dma_start(out=out_d[tok, :], in_=oo)

        # ---------------- orth-loss partial (sum of squares) ----------------
        with nc.named_scope("ss"):
            ssp = wall.tile([P, 1], F32)
            nc.vector.tensor_reduce(ssp, ss_all, axis=AX.X, op=ALU.add)
            pss = psum_h.tile([1, 1], F32, tag="h")
            nc.tensor.matmul(pss, ssp, ones_t, start=True, stop=True)
            sst = wall.tile([1, 1], F32)
            nc.scalar.copy(sst, pss)
            nc.sync.dma_start(out=ss_d[:], in_=sst)

    nc.compile()
    return nc


@lru_cache(maxsize=2)
def _get_program(trivia_key):
    return _build(dict(trivia_key))


def kernel(x, Wg, bg, W1, b1, ln1_w, ln1_b, W2, b2, ln2_w, ln2_b, k):
    assert int(k) == 2, "kernel specialized for k=2"
    x = np.asarray(x, np.float32)
    Wg = np.asarray(Wg, np.float32)
    bg = np.asarray(bg, np.float32)
    W1 = np.asarray(W1, np.float32)
    W2 = np.asarray(W2, np.float32)
    b1 = np.asarray(b1, np.float32)
    b2 = np.asarray(b2, np.float32)
    ln1_w = np.asarray(ln1_w, np.float32)
    ln1_b = np.asarray(ln1_b, np.float32)
    ln2_w = np.asarray(ln2_w, np.float32)
    ln2_b = np.asarray(ln2_b, np.float32)

    trivia = {
        "b1": not b1.any(),
        "b2": not b2.any(),
        "ln1_w": bool((ln1_w == 1.0).all()),
        "ln1_b": not ln1_b.any(),
        "ln2_w": bool((ln2_w == 1.0).all()),
        "ln2_b": not ln2_b.any(),
    }
    nc = _get_program(tuple(sorted(trivia.items())))

    # host-side shard/layout prep (pure data marshalling)
    w1h = np.ascontiguousarray(
        W1.reshape(E, KC, P, D).transpose(2, 0, 1, 3)).astype(BF16_NP)
    w2h = np.ascontiguousarray(
        W2.reshape(E, KC, P, D).transpose(2, 0, 1, 3)).astype(BF16_NP)
    wgh = np.ascontiguousarray(Wg.reshape(KC, P, E).transpose(1, 0, 2))
    bgh = bg.reshape(1, E)

    in_maps = []
    for b in range(B):
        xb = x[b]                                   # (N, D)
        xtf = np.ascontiguousarray(
            xb.T.reshape(KC, P, N).transpose(1, 0, 2))   # (P, KC, N)
        m = {
            "xt_f": xtf,
            "xt_b": xtf.astype(BF16_NP),
            "x_f": xb,
            "w1": w1h,
            "w2": w2h,
            "wg": wgh,
            "bg": bgh,
        }
        for name, arr in (("b1", b1), ("ln1_w", ln1_w), ("ln1_b", ln1_b),
                          ("b2", b2), ("ln2_w", ln2_w), ("ln2_b", ln2_b)):
            if not trivia[name]:
                m[name] = arr
        in_maps.append(m)

    trace = bool(int(os.environ.get("KBENCH_TRACE", "0")))
    if trace:
        res = run_bass_kernel_spmd(nc, in_maps, list(range(B)), trace=True)
        kernel.last_exec_time_ns = res.exec_time_ns
        kernel.last_trace = res.instructions_and_trace
        kernel.last_mean_exec_time_ns = res.mean_exec_time_ns
    else:
        res = run_bass_kernel_spmd(nc, in_maps, list(range(B)))

    out = np.stack([res.results[b]["out"] for b in range(B)])
    out_top = np.stack([res.results[b]["out_top"] for b in range(B)])
    out_bot = np.stack([res.results[b]["out_bot"] for b in range(B)])
    ss = np.array([res.results[b]["ss"][0, 0] for b in range(B)], np.float32)
    orth = np.float32(np.mean(1.0 / (np.sqrt(ss) + 1e-8)))
    return out, out_top, out_bot, orth
